# revision 1
# baseline (speedup 1.0000x reference)
# Trainium2 Bass kernel for nn_Block_9483287789889 (dense transformer block).
#
# Sharding (8 cores): 2 cores per batch (B=4). Host permutes each batch's
# 2048 tokens into [owned 8x128-tiles (interleaved) | other 8 tiles] so both
# cores of a pair run an IDENTICAL program (SPMD) with all per-core variation
# carried by input data (token permutation + boundary-mask patterns).
# Attention (softmax over the QUERY axis -> per-key normalizers Z[s]) is
# computed in S^T layout [s_partition, t_free]; Z comes free from ACT's
# activation(exp, accum_out=...). Each core computes exp over all quadrants
# it needs for full-local Z (split_z=False) or only its owned-query quadrants
# plus a 2-core AllReduce of Z partials (split_z=True).
import sys

if "/opt/trn_rl_repo" not in sys.path:
    sys.path.insert(0, "/opt/trn_rl_repo")

import numpy as np
import ml_dtypes

BF16 = ml_dtypes.bfloat16

B, T, C, H, HS = 4, 2048, 384, 6, 64
D4 = 4 * C  # 1536
EPS = 1e-5
NPAIR = H // 2  # 3 head-pairs
P = 128
NT = T // P  # 16 token tiles
CH = 512
NCH = T // CH  # 4 chunks of 512
OWN = T // 2  # 1024 owned tokens per core
NEG = -30.0
SCALE = float(C) ** -0.5
N_CORES = 8

_PROG_CACHE = {}


def _slot_table(c):
    """Slots for local t-chunk c (c in 0..3). Returns list of dicts.

    Local layout: t-tiles 0..7 = owned (interleaved phys), 8..15 = other.
    Chunks 0,1 cover owned tiles (u_t = 4c+jj); chunks 2,3 cover other tiles
    (u_t = 4(c-2)+jj). Quadrants:
      q1: s owned vs t owned  -> causal u_s <= u_t, true diag -> tril mask
      q3: s other vs t owned  -> block-causal, boundary block all-or-nothing
      q2: s owned vs t other  -> block-causal, boundary all-or-nothing
      q4: s other vs t other  -> causal, true diag tril
    """
    cc = c if c < 2 else c - 2
    quads = [("q1", 0, "tril"), ("q3", 8, "q3m")] if c < 2 else [
        ("q2", 0, "q2m"), ("q4", 8, "tril")]
    slots = []
    for quad, s_base, mpat in quads:
        for k in range(4 * cc + 4):
            sub0 = max(0, k - 4 * cc)
            w = CH - P * sub0
            # boundary/diag sub-block present iff k >= 4*cc
            has_mask = k >= 4 * cc
            slots.append(dict(quad=quad, s_tile=s_base + k, k=k, sub0=sub0,
                              w=w, mask=(mpat if has_mask else None)))
    return slots


def _pt_layout():
    """Column offsets of stored P^T slots (owned chunks only) per head."""
    off = {}
    pos = 0
    for c in (0, 1):
        for s in _slot_table(c):
            off[(c, s["quad"], s["k"])] = (pos, s["w"])
            pos += s["w"]
    return off, pos  # pos = total columns per (pair, head)


def _build_program(split_z, nz, taps=False):
    """nz: dict of nonzero-bias flags (bqk, bv, bproj, b2)."""
    import concourse.bass as bass
    import concourse.bacc as bacc
    import concourse.mybir as mybir
    from concourse.tile import TileContext
    from contextlib import ExitStack

    f32 = mybir.dt.float32
    bf16 = mybir.dt.bfloat16
    AF = mybir.ActivationFunctionType
    ALU = mybir.AluOpType

    nc = bacc.Bacc("TRN2", target_bir_lowering=False)

    x_d = nc.dram_tensor("x_perm", [T, C], f32, kind="ExternalInput")
    wqk_d = nc.dram_tensor("wqk", [P, 3, NPAIR, 2, P], bf16, kind="ExternalInput")
    wv_d = nc.dram_tensor("wv", [P, 3, C], bf16, kind="ExternalInput")
    wp_d = nc.dram_tensor("wp", [P, 3, C], bf16, kind="ExternalInput")
    w1_d = nc.dram_tensor("w1", [P, 3, D4], bf16, kind="ExternalInput")
    w2_d = nc.dram_tensor("w2", [P, 12, C], bf16, kind="ExternalInput")
    b1_d = nc.dram_tensor("b1", [P, 12], f32, kind="ExternalInput")
    ident_d = nc.dram_tensor("ident", [P, P], bf16, kind="ExternalInput")
    negi_d = nc.dram_tensor("negi", [P, P], bf16, kind="ExternalInput")
    # mask patterns (rhs of -30*I @ pattern accumulated into S psum):
    # tril: strict-lower ones (const); q3m/q2m: all-ones or all-zeros per core
    tril_d = nc.dram_tensor("trilm", [P, P], bf16, kind="ExternalInput")
    q3m_d = nc.dram_tensor("q3m", [P, P], bf16, kind="ExternalInput")
    q2m_d = nc.dram_tensor("q2m", [P, P], bf16, kind="ExternalInput")
    if nz["bqk"]:
        bqk_d = nc.dram_tensor("bqk", [P, NPAIR, 2], f32, kind="ExternalInput")
    if nz["bv"]:
        bv_d = nc.dram_tensor("bv", [P, C], f32, kind="ExternalInput")
    if nz["bproj"]:
        bproj_d = nc.dram_tensor("bproj", [P, C], f32, kind="ExternalInput")
    if nz["b2"]:
        b2_d = nc.dram_tensor("b2", [P, C], f32, kind="ExternalInput")
    if split_z:
        swapsel_d = nc.dram_tensor("swapsel", [P, 16], mybir.dt.uint8,
                                   kind="ExternalInput")
        zin_d = nc.dram_tensor("zin", [NPAIR, 2, P, 16], f32)
        zout_d = nc.dram_tensor("zout", [NPAIR, 2, P, 16], f32)
    out_d = nc.dram_tensor("out", [OWN, C], f32, kind="ExternalOutput")
    tap_d = {}
    if taps:
        for nm, shp, dt in [("hT", [P, 3, T], bf16), ("v", [P, NT, C], bf16),
                            ("qt0", [P, T], bf16), ("kt0", [P, T], bf16),
                            ("pt00", [P, 9216], bf16), ("pt01", [P, 9216], bf16),
                            ("z00", [P, 16], f32), ("z01", [P, 16], f32),
                            ("att", [P, NPAIR, OWN], bf16),
                            ("r", [P, 8, C], f32), ("h2T", [P, 3, OWN], bf16)]:
            tap_d[nm] = nc.dram_tensor("tap_" + nm, shp, dt,
                                       kind="ExternalOutput")

    pt_off, pt_cols = _pt_layout()

    with TileContext(nc) as tc, ExitStack() as ctx:
        pool1 = ctx.enter_context(tc.tile_pool(name="const", bufs=1))
        persist = ctx.enter_context(tc.tile_pool(name="persist", bufs=1))
        lnp = ctx.enter_context(tc.tile_pool(name="ln", bufs=4))
        qkp = ctx.enter_context(tc.tile_pool(name="qk", bufs=2))
        ptp = ctx.enter_context(tc.tile_pool(name="ptp", bufs=2))
        zp = ctx.enter_context(tc.tile_pool(name="zp", bufs=3))
        scr = ctx.enter_context(tc.tile_pool(name="scr", bufs=4))
        hidp = ctx.enter_context(tc.tile_pool(name="hid", bufs=13))
        outp = ctx.enter_context(tc.tile_pool(name="outp", bufs=8))
        ps_s = ctx.enter_context(tc.tile_pool(name="ps_s", bufs=2, space="PSUM"))
        ps_pv = ctx.enter_context(tc.tile_pool(name="ps_pv", bufs=2, space="PSUM"))
        ps_mm = ctx.enter_context(tc.tile_pool(name="ps_mm", bufs=2, space="PSUM"))
        ps_tr = ctx.enter_context(tc.tile_pool(name="ps_tr", bufs=2, space="PSUM"))

        # ---- constants / weights into SBUF
        wqk_sb = pool1.tile([P, 3, NPAIR, 2, P], bf16)
        nc.gpsimd.dma_start(out=wqk_sb, in_=wqk_d[:])
        wv_sb = pool1.tile([P, 3, C], bf16)
        nc.gpsimd.dma_start(out=wv_sb, in_=wv_d[:])
        wp_sb = pool1.tile([P, 3, C], bf16)
        nc.gpsimd.dma_start(out=wp_sb, in_=wp_d[:])
        w1_sb = pool1.tile([P, 3, D4], bf16)
        nc.gpsimd.dma_start(out=w1_sb, in_=w1_d[:])
        w2_sb = pool1.tile([P, 12, C], bf16)
        nc.gpsimd.dma_start(out=w2_sb, in_=w2_d[:])
        b1_sb = pool1.tile([P, 12], f32)
        nc.gpsimd.dma_start(out=b1_sb, in_=b1_d[:])
        ident_sb = pool1.tile([P, P], bf16)
        nc.gpsimd.dma_start(out=ident_sb, in_=ident_d[:])
        negi_sb = pool1.tile([P, P], bf16)
        nc.gpsimd.dma_start(out=negi_sb, in_=negi_d[:])
        mask_sb = {}
        for nm, d in (("tril", tril_d), ("q3m", q3m_d), ("q2m", q2m_d)):
            m = pool1.tile([P, P], bf16, name=f"m_{nm}")
            nc.gpsimd.dma_start(out=m, in_=d[:])
            mask_sb[nm] = m
        if nz["bqk"]:
            bqk_sb = pool1.tile([P, NPAIR, 2], f32)
            nc.gpsimd.dma_start(out=bqk_sb, in_=bqk_d[:])
        if nz["bv"]:
            bv_sb = pool1.tile([P, C], f32)
            nc.gpsimd.dma_start(out=bv_sb, in_=bv_d[:])
        if nz["bproj"]:
            bproj_sb = pool1.tile([P, C], f32)
            nc.gpsimd.dma_start(out=bproj_sb, in_=bproj_d[:])
        if nz["b2"]:
            b2_sb = pool1.tile([P, C], f32)
            nc.gpsimd.dma_start(out=b2_sb, in_=b2_d[:])
        if split_z:
            swapsel_sb = pool1.tile([P, 16], mybir.dt.uint8)
            nc.gpsimd.dma_start(out=swapsel_sb, in_=swapsel_d[:])

        eps_sb = pool1.tile([P, 1], f32)
        nc.vector.memset(eps_sb, EPS)

        x_sb = persist.tile([P, NT, C], f32)
        nc.gpsimd.dma_start(out=x_sb,
                            in_=x_d.rearrange("(n p) c -> p n c", p=P))
        hT = persist.tile([P, 3, T], bf16)       # normalized x, transposed
        v_sb = persist.tile([P, NT, C], bf16)    # V (later scaled to V/Z)
        att_sb = persist.tile([P, NPAIR, OWN], bf16)  # attention out^T
        h2T = persist.tile([P, 3, OWN], bf16)    # LN2 out, transposed
        r_sb = persist.tile([P, 8, C], f32)      # residual-1 tiles (owned)

        # ---- LN1 + transpose into hT
        for i in range(NT):
            x_t = x_sb[:, i, :]
            st = lnp.tile([P, 6], f32, name="st")
            nc.vector.bn_stats(out=st, in_=x_t)
            mv = lnp.tile([P, 2], f32, name="mv")
            nc.vector.bn_aggr(out=mv, in_=st)
            rs = lnp.tile([P, 1], f32, name="rs")
            nc.scalar.activation(out=rs, in_=mv[:, 1:2], func=AF.Sqrt, bias=eps_sb)
            nc.vector.reciprocal(out=rs, in_=rs)
            hb = lnp.tile([P, C], bf16, name="hb")
            nc.vector.tensor_scalar(out=hb, in0=x_t, scalar1=mv[:, 0:1],
                                    scalar2=rs, op0=ALU.subtract, op1=ALU.mult)
            for cc in range(3):
                tp = ps_tr.tile([P, P], bf16, name="tp")
                nc.tensor.transpose(tp, hb[:, cc * P:(cc + 1) * P], ident_sb)
                nc.any.tensor_copy(hT[:, cc, i * P:(i + 1) * P], tp)

        # ---- V for all heads (lhsT = hT chunk, rhs = wv)
        for i in range(NT):
            pv = ps_mm.tile([P, C], f32, name="pv", tag="pq")
            for cc in range(3):
                nc.tensor.matmul(pv, hT[:, cc, i * P:(i + 1) * P],
                                 wv_sb[:, cc, :], start=(cc == 0), stop=(cc == 2))
            if nz["bv"]:
                nc.vector.tensor_add(out=v_sb[:, i, :], in0=pv, in1=bv_sb)
            else:
                nc.any.tensor_copy(v_sb[:, i, :], pv)

        if taps:
            nc.gpsimd.dma_start(out=tap_d["hT"][:], in_=hT)
            nc.gpsimd.dma_start(out=tap_d["v"][:], in_=v_sb)

        # ---- per-pair attention
        qt = {}
        kt = {}
        ptt = {}
        zloc = {}
        for p in range(NPAIR):
            qt[p] = qkp.tile([P, T], bf16, name=f"qt{p}", tag="qt")
            kt[p] = qkp.tile([P, T], bf16, name=f"kt{p}", tag="kt")
            for c in range(NCH):
                for qk, dst in ((0, qt[p]), (1, kt[p])):
                    pq = ps_mm.tile([P, CH], f32, name="pq")
                    for cc in range(3):
                        nc.tensor.matmul(pq, wqk_sb[:, cc, p, qk, :],
                                         hT[:, cc, c * CH:(c + 1) * CH],
                                         start=(cc == 0), stop=(cc == 2))
                    if nz["bqk"]:
                        nc.vector.tensor_scalar(
                            out=dst[:, c * CH:(c + 1) * CH], in0=pq,
                            scalar1=bqk_sb[:, p, qk:qk + 1], scalar2=None,
                            op0=ALU.add)
                    else:
                        nc.any.tensor_copy(dst[:, c * CH:(c + 1) * CH], pq)

            # pass 1: S^T tiles -> exp -> P^T cache + Z partials
            for h in range(2):
                ptt[(p, h)] = ptp.tile([P, pt_cols], bf16,
                                       name=f"pt{p}_{h}", tag="pt")
                zloc[(p, h)] = zp.tile([P, 16, 4], f32,
                                       name=f"zs{p}_{h}", tag="zs")
                nc.vector.memset(zloc[(p, h)], 0.0)
            chunks = (0, 1) if split_z else (0, 1, 2, 3)
            for c in chunks:
                for s in _slot_table(c):
                    for h in range(2):
                        hb_ = h * 64
                        spsum = ps_s.tile([P, CH], f32, name="spsum", tag="sp")
                        nc.tensor.matmul(
                            spsum[:, s["sub0"] * P:],
                            kt[p][hb_:hb_ + 64, s["s_tile"] * P:(s["s_tile"] + 1) * P],
                            qt[p][hb_:hb_ + 64, c * CH + s["sub0"] * P:(c + 1) * CH],
                            start=True, stop=(s["mask"] is None))
                        if s["mask"] is not None:
                            bs = s["k"] - 4 * (c if c < 2 else c - 2)
                            nc.tensor.matmul(
                                spsum[:, bs * P:(bs + 1) * P],
                                negi_sb, mask_sb[s["mask"]],
                                start=False, stop=True)
                        zcol = zloc[(p, h)][:, s["s_tile"], c:c + 1]
                        if c < 2:
                            o, w = pt_off[(c, s["quad"], s["k"])]
                            dst = ptt[(p, h)][:, o:o + w]
                        else:
                            sc = scr.tile([P, CH], bf16, name="sc", tag="sc")
                            dst = sc[:, :s["w"]]
                        nc.scalar.activation(out=dst, in_=spsum[:, s["sub0"] * P:],
                                             func=AF.Exp, accum_out=zcol)

            if taps and p == 0:
                nc.gpsimd.dma_start(out=tap_d["qt0"][:], in_=qt[p])
                nc.gpsimd.dma_start(out=tap_d["kt0"][:], in_=kt[p])
                nc.gpsimd.dma_start(out=tap_d["pt00"][:], in_=ptt[(p, 0)])
                nc.gpsimd.dma_start(out=tap_d["pt01"][:], in_=ptt[(p, 1)])

            # Z combine (+ AllReduce when split across the core pair)
            zfin = {}
            for h in range(2):
                zl = zp.tile([P, 16], f32, name=f"zl{p}_{h}", tag="zl")
                nc.vector.tensor_reduce(out=zl, in_=zloc[(p, h)],
                                        axis=mybir.AxisListType.X, op=ALU.add)
                zfin[h] = zl
                if split_z:
                    # local->canonical order: swap 8-col halves iff swapsel=1
                    zc = zp.tile([P, 16], f32, name=f"zc{p}_{h}", tag="zc")
                    nc.vector.tensor_copy(zc, zl)
                    zsw = zp.tile([P, 16], f32, name=f"zw{p}_{h}", tag="zw")
                    nc.vector.tensor_copy(zsw[:, 0:8], zl[:, 8:16])
                    nc.vector.tensor_copy(zsw[:, 8:16], zl[:, 0:8])
                    nc.vector.copy_predicated(zc, swapsel_sb, zsw)
                    nc.gpsimd.dma_start(out=zin_d[p, h], in_=zc)
            if split_z:
                nc.gpsimd.collective_compute(
                    "AllReduce", ALU.add,
                    replica_groups=[[0, 1], [2, 3], [4, 5], [6, 7]],
                    ins=[zin_d[p, :, :, :]], outs=[zout_d[p, :, :, :]])
                for h in range(2):
                    zs = zp.tile([P, 16], f32, name=f"zg{p}_{h}", tag="zg",
                                 bufs=6)
                    nc.gpsimd.dma_start(out=zs, in_=zout_d[p, h])
                    # canonical->local: same conditional swap
                    zsw2 = zp.tile([P, 16], f32, name=f"zx{p}_{h}", tag="zx")
                    nc.vector.tensor_copy(zsw2[:, 0:8], zs[:, 8:16])
                    nc.vector.tensor_copy(zsw2[:, 8:16], zs[:, 0:8])
                    nc.vector.copy_predicated(zs, swapsel_sb, zsw2)
                    zfin[h] = zs
            for h in range(2):
                zl = zfin[h]
                if taps and p == 0:
                    nc.gpsimd.dma_start(out=tap_d[f"z0{h}"][:], in_=zl)
                nc.vector.reciprocal(out=zl, in_=zl)
                for k in range(16):
                    col = (2 * p + h) * 64
                    nc.vector.tensor_scalar_mul(
                        out=v_sb[:, k, col:col + 64],
                        in0=v_sb[:, k, col:col + 64], scalar1=zl[:, k:k + 1])

            # pass 2: out^T = sum_s (V/Z)^T-slices @ P^T
            for c in (0, 1):
                pvp = ps_pv.tile([P, CH], f32, name="pvp", tag="pvp")
                slots = _slot_table(c)
                # all of head 0, then all of head 1, each opening with
                # start=True: correct regardless of whether the has_written
                # clear is whole-bank or per-region.
                for h in range(2):
                    for n, s in enumerate(slots):
                        o, w = pt_off[(c, s["quad"], s["k"])]
                        nc.tensor.matmul(
                            pvp[h * 64:(h + 1) * 64, s["sub0"] * P:],
                            v_sb[:, s["s_tile"], (2 * p + h) * 64:(2 * p + h + 1) * 64],
                            ptt[(p, h)][:, o:o + w],
                            start=(n == 0), stop=(n == len(slots) - 1),
                            tile_position=(0, h * 64))
                nc.any.tensor_copy(att_sb[:, p, c * CH:(c + 1) * CH], pvp)

        if taps:
            nc.gpsimd.dma_start(out=tap_d["att"][:], in_=att_sb)

        # ---- projection + residual 1
        for i in range(8):
            py = ps_mm.tile([P, C], f32, name="py", tag="pq")
            for p in range(NPAIR):
                nc.tensor.matmul(py, att_sb[:, p, i * P:(i + 1) * P],
                                 wp_sb[:, p, :], start=(p == 0), stop=(p == 2))
            nc.vector.tensor_add(out=r_sb[:, i, :], in0=py, in1=x_sb[:, i, :])
            if nz["bproj"]:
                nc.vector.tensor_add(out=r_sb[:, i, :], in0=r_sb[:, i, :],
                                     in1=bproj_sb)

        # ---- LN2 + transpose
        for i in range(8):
            st2 = lnp.tile([P, 6], f32, name="st2")
            nc.vector.bn_stats(out=st2, in_=r_sb[:, i, :])
            mv2 = lnp.tile([P, 2], f32, name="mv2")
            nc.vector.bn_aggr(out=mv2, in_=st2)
            rs2 = lnp.tile([P, 1], f32, name="rs2")
            nc.scalar.activation(out=rs2, in_=mv2[:, 1:2], func=AF.Sqrt, bias=eps_sb)
            nc.vector.reciprocal(out=rs2, in_=rs2)
            h2b = lnp.tile([P, C], bf16, name="h2b")
            nc.vector.tensor_scalar(out=h2b, in0=r_sb[:, i, :],
                                    scalar1=mv2[:, 0:1], scalar2=rs2,
                                    op0=ALU.subtract, op1=ALU.mult)
            for cc in range(3):
                tp2 = ps_tr.tile([P, P], bf16, name="tp2", tag="tp")
                nc.tensor.transpose(tp2, h2b[:, cc * P:(cc + 1) * P], ident_sb)
                nc.any.tensor_copy(h2T[:, cc, i * P:(i + 1) * P], tp2)

        if taps:
            nc.gpsimd.dma_start(out=tap_d["r"][:], in_=r_sb)
            nc.gpsimd.dma_start(out=tap_d["h2T"][:], in_=h2T)

        # ---- FFN + residual 2 + store
        for c in range(2):
            hid = []
            for cb in range(12):
                ph = ps_mm.tile([P, CH], f32, name="ph", tag="pq")
                for cc in range(3):
                    nc.tensor.matmul(ph, w1_sb[:, cc, cb * P:(cb + 1) * P],
                                     h2T[:, cc, c * CH:(c + 1) * CH],
                                     start=(cc == 0), stop=(cc == 2))
                ht_ = hidp.tile([P, CH], bf16, name=f"ht{c}_{cb}", tag="hid")
                nc.vector.tensor_scalar(out=ht_, in0=ph, scalar1=b1_sb[:, cb:cb + 1],
                                        scalar2=0.0, op0=ALU.add, op1=ALU.max)
                hid.append(ht_)
            for jj in range(4):
                i = c * 4 + jj
                pf = ps_mm.tile([P, C], f32, name="pf", tag="pq")
                for cb in range(12):
                    nc.tensor.matmul(pf, hid[cb][:, jj * P:(jj + 1) * P],
                                     w2_sb[:, cb, :], start=(cb == 0),
                                     stop=(cb == 11))
                ot = outp.tile([P, C], f32, name="ot")
                nc.vector.tensor_add(out=ot, in0=pf, in1=r_sb[:, i, :])
                if nz["b2"]:
                    nc.vector.tensor_add(out=ot, in0=ot, in1=b2_sb)
                nc.gpsimd.dma_start(out=out_d[i * P:(i + 1) * P, :], in_=ot)

    nc.compile()
    return nc


def _prep_inputs(inputs, split_z):
    """Host-side: fold gains into weights, build per-core input maps."""
    x = np.asarray(inputs["x"], np.float32)
    g1 = np.asarray(inputs["g1"], np.float32)
    be1 = np.asarray(inputs["be1"], np.float32)
    g2 = np.asarray(inputs["g2"], np.float32)
    be2 = np.asarray(inputs["be2"], np.float32)
    # attention scale folded into wq so masks added to S psum stay at NEG
    wq = np.asarray(inputs["wq"], np.float32) * g1[None, :, None] * SCALE
    wk = np.asarray(inputs["wk"], np.float32) * g1[None, :, None]
    wv = np.asarray(inputs["wv"], np.float32) * g1[None, :, None]
    bq = np.einsum("c,hcd->hd", be1,
                   np.asarray(inputs["wq"], np.float32)) * SCALE
    bk = np.einsum("c,hcd->hd", be1, np.asarray(inputs["wk"], np.float32))
    bv = np.einsum("c,hcd->hd", be1, np.asarray(inputs["wv"], np.float32))
    wp = np.asarray(inputs["w_proj"], np.float32)
    bproj = np.asarray(inputs["b_proj"], np.float32)
    w1 = np.asarray(inputs["w1"], np.float32) * g2[:, None]
    b1 = np.asarray(inputs["b1"], np.float32) + be2 @ np.asarray(
        inputs["w1"], np.float32)
    w2 = np.asarray(inputs["w2"], np.float32)
    b2 = np.asarray(inputs["b2"], np.float32)

    nz = dict(bqk=bool(np.any(bq) or np.any(bk)), bv=bool(np.any(bv)),
              bproj=bool(np.any(bproj)), b2=bool(np.any(b2)))

    # wqk [128, cc, pair, qk, col]: lhsT chunks (c-partition, head-pair cols)
    wqk = np.zeros((P, 3, NPAIR, 2, P), BF16)
    for pr in range(NPAIR):
        for qk, w in ((0, wq), (1, wk)):
            pair = np.concatenate([w[2 * pr], w[2 * pr + 1]], axis=1)  # [C,128]
            wqk[:, :, pr, qk, :] = pair.reshape(3, P, P).transpose(1, 0, 2)
    wv_all = np.concatenate([wv[h] for h in range(H)], axis=1)  # [C, 384]
    wv_pre = wv_all.reshape(3, P, C).transpose(1, 0, 2).astype(BF16)
    wp_pre = wp.reshape(3, P, C).transpose(1, 0, 2).astype(BF16)
    w1_pre = w1.reshape(3, P, D4).transpose(1, 0, 2).astype(BF16)
    w2_pre = w2.reshape(12, P, C).transpose(1, 0, 2).astype(BF16)
    b1_pre = np.ascontiguousarray(b1.reshape(12, P).T).astype(np.float32)

    ident = np.eye(P, dtype=BF16)
    negi = (np.eye(P) * NEG).astype(BF16)
    sl = np.tril(np.ones((P, P)), -1).astype(BF16)  # strict lower: s > t

    common = dict(wqk=wqk, wv=wv_pre, wp=wp_pre, w1=w1_pre, w2=w2_pre,
                  b1=b1_pre, ident=ident, negi=negi, trilm=sl)
    if nz["bqk"]:
        bqk = np.zeros((P, NPAIR, 2), np.float32)
        for pr in range(NPAIR):
            bqk[:, pr, 0] = np.concatenate([bq[2 * pr], bq[2 * pr + 1]])
            bqk[:, pr, 1] = np.concatenate([bk[2 * pr], bk[2 * pr + 1]])
        common["bqk"] = bqk
    if nz["bv"]:
        common["bv"] = np.broadcast_to(
            np.concatenate([bv[h] for h in range(H)]), (P, C)).copy()
    if nz["bproj"]:
        common["bproj"] = np.broadcast_to(bproj, (P, C)).copy()
    if nz["b2"]:
        common["b2"] = np.broadcast_to(b2, (P, C)).copy()

    ones = np.ones((P, P), BF16)
    zeros = np.zeros((P, P), BF16)
    in_maps = []
    perms = []
    for core in range(N_CORES):
        b, par = core // 2, core % 2
        perm = list(range(par, NT, 2)) + list(range(1 - par, NT, 2))
        perms.append(perm)
        xt = x[b].reshape(NT, P, C)[perm].reshape(T, C).astype(np.float32)
        m = dict(common)
        m["x_perm"] = xt
        # q3 boundary (s other, t owned): phys 2u+ (1-par) vs 2u+par:
        #   par=0: s odd > t even at boundary -> invalid -> mask ON
        m["q3m"] = ones if par == 0 else zeros
        m["q2m"] = zeros if par == 0 else ones
        if split_z:
            m["swapsel"] = np.full((P, 16), par, np.uint8)
        in_maps.append(m)
    return in_maps, perms, nz


def _purge_neff_cache():
    # libneuronxla's NEFF cache is keyed on the HLO module hash, which does
    # not cover the BIR carried in backend_config -- a stale kernel body can
    # be silently reused across program edits. Purge before compiling.
    import glob, os, shutil
    for d in glob.glob(os.path.expanduser(
            "~/.neuron-compile-cache/*/MODULE_*")):
        try:
            shutil.rmtree(d, ignore_errors=True)
        except OSError:
            pass


def kernel(**inputs):
    import os
    split_z = os.environ.get("KSPLITZ", "1") == "1"
    _purge_neff_cache()
    in_maps, perms, nz = _prep_inputs(inputs, split_z)
    key = (split_z, tuple(sorted(nz.items())))
    if key not in _PROG_CACHE:
        _PROG_CACHE[key] = _build_program(split_z, nz)
    nc = _PROG_CACHE[key]

    from concourse.bass_utils import run_bass_kernel_spmd
    res = run_bass_kernel_spmd(nc, in_maps, core_ids=list(range(N_CORES)))

    out = np.empty((B, T, C), np.float32)
    for core in range(N_CORES):
        b, par = core // 2, core % 2
        tiles = out[b].reshape(NT, P, C)
        tiles[par::2] = res.results[core]["out"].reshape(8, P, C)
    return out

